# revision 35
# baseline (speedup 1.0000x reference)
"""Multi-head attention (B=4, S=2048, H=1024, NH=16) on 8 trn2 NeuronCores.

Sharding: token-parallel, no collectives. Core c handles batch b=c//2,
query half h=c%2 (1024 query tokens), with the full 2048-key K/V of its
batch (K/V projection duplicated within each core pair).

Per-core pipeline (bf16 matmul inputs, fp32 PSUM accumulation):
  A) Q projection (up front) -> SBUF-resident bf16 pair slabs Qp[og]
     [128, 1024] (rows 0:64 = head 2og features, 64:128 = head 2og+1).
     K projection -> feature-major pair slabs projected one pair ahead
     directly into rotating SBUF tiles (ktsl, 2 bufs) -- no DRAM spill.
     The 1/8 attention scale is folded into Wq on the host; Q/K biases
     are added during the DVE PSUM evacuation.
  B) QK^T via 64x128 PE row tiling: per (pair, kc, qg) one PSUM tile
     [128 ktok, 1024] whose halves are written by two concurrent K=64
     matmuls (tile T0 = head 2p from partitions 0:64, tile T8 = head
     2p+1 from partitions 64:128) -- no zero padding, 2 cols/cycle
     aggregate.  One fused ScalarE exp per tile (constant bias -ln8,
     mask NOT applied here) emits E bf16.
  C) The attention mask is applied multiplicatively on the V side: the
     host passes emask = exp((1-mask)*-10000) per key token and the V
     projection PSUM evacuation scales each token row (including the
     per-head denominator column) by it.  V output lives in per-quarter
     tiles Vq[kc] [128, 260] (4 heads), double-buffered so only two
     quarters are SBUF-live at a time.
  D) PV: per (head, query-pair) chains whose per-key-chunk matmuls are
     interleaved into the same phase's qkt/exp slot stream (trailing the
     exps by ~2 slots, which caps live E tiles at ~6), accumulating in
     65-wide slots packed 4-per-PSUM-bank; only the first chain per bank
     issues start=True (start clears the whole bank's has_written bits).
     Normalization by the denominator column happens at evacuation.
  E) Output projection: ctx chunk i depends only on head-pair i, so
     chunks 0-6 are transposed (PE, 8-slot PSUM bank) and accumulated
     during pair 7's idle slots with the partial spilled to SBUF; only
     the chunk-7 term + bias trail the last exp.  A ~4us dummy-matmul
     spinner at the head opens the HAM clock gate before the warmup
     chain.  DMA out [1024, 1024] fp32.
"""

import math

import numpy as np
import ml_dtypes

import concourse.tile as tile
from concourse import bacc, mybir
from concourse.bass_utils import run_bass_kernel_spmd
from concourse.masks import make_identity

B, S, H, NH, HDIM = 4, 2048, 1024, 16, 64
NCORES = 8
TOK = 1024            # query tokens per core
KTOK = 2048           # key tokens per core
IC = H // 128         # 8 feature chunks of 128
KC = KTOK // 128      # 16 key chunks of 128
QT = TOK // 128       # 8 query tiles of 128
NP = NH // 2          # 8 head pairs
VW = NH * (HDIM + 1)  # 1040: V columns incl. per-head denominator column
VQ = VW // 4          # 260: V column quarter = 4 heads
BF = mybir.dt.bfloat16
F32 = mybir.dt.float32
E_BUFS = 21
LN8 = math.log(8.0)

_CACHE = {}


def _emit(nc, tc, io):
    Exp = mybir.ActivationFunctionType.Exp

    persist = tc.alloc_tile_pool(name="persist", bufs=1)
    psum = tc.alloc_tile_pool(name="psum", bufs=2, space="PSUM")
    attnp = tc.alloc_tile_pool(name="attnp", bufs=1)

    # ---- persistent tiles ----
    ones = persist.tile([1, 128], BF, name="ones", tag="ones")
    nc.vector.memset(ones[:], 1.0)
    ident = persist.tile([128, 128], BF, name="ident", tag="ident")
    make_identity(nc, ident[:])
    emask = persist.tile([128, KC], F32, name="emask", tag="emask")
    nc.sync.dma_start(emask[:], io["emaskcol"][:])
    bqc = persist.tile([128, IC], F32, name="bqc", tag="bqc")
    nc.sync.dma_start(bqc[:], io["bqcol"][:])
    bkc = persist.tile([128, IC], F32, name="bkc", tag="bkc")
    nc.sync.dma_start(bkc[:], io["bkcol"][:])
    nln8 = persist.tile([128, 1], F32, name="nln8", tag="nln8")
    nc.vector.memset(nln8[:], -LN8)
    # HAM warm-up: ~4us of dummy matmuls while the input DMAs stream, so
    # the PE clock-gate opens to 2.4 GHz before the real warmup chain.
    warm = psum.tile([128, 128], F32, name="warm", tag="proj")
    for _ in range(40):
        nc.tensor.matmul(warm[:], ones[0:1, 0:128], ones[0:1, 0:128],
                         start=True, stop=True)

    Qp = [persist.tile([128, TOK], BF, name=f"Qp{i}", tag=f"Qp{i}")
          for i in range(IC)]
    ctx = [persist.tile([128, H], BF, name=f"ctx{i}", tag=f"ctx{i}")
           for i in range(QT)]

    # ---- Q projection: up front, into resident pair slabs ----
    q_pool = {}

    def open_q_pool():
        ap = tc.alloc_tile_pool(name="q_pool", bufs=1)
        q_pool["pool"] = ap
        q_pool["x"] = []
        q_pool["w"] = []
        for i in range(IC):
            x = ap.tile([128, TOK], BF, name=f"q_x{i}", tag=f"qx{i}")
            nc.sync.dma_start(x[:], io["qT"][i * 128:(i + 1) * 128, :])
            w = ap.tile([128, H], BF, name=f"q_w{i}", tag=f"qw{i}")
            nc.sync.dma_start(w[:], io["wqT"][i * 128:(i + 1) * 128, :])
            q_pool["x"].append(x)
            q_pool["w"].append(w)

    def emit_q_og(og):
        x_s, w_s = q_pool["x"], q_pool["w"]
        pa = psum.tile([128, 512], F32, name="ps_qa", tag="proj")
        pb = psum.tile([128, 512], F32, name="ps_qb", tag="proj")
        for i in range(IC):
            w = w_s[i][:, og * 128:(og + 1) * 128]
            nc.tensor.matmul(pa[:], w, x_s[i][:, 0:512],
                             start=(i == 0), stop=(i == IC - 1))
            nc.tensor.matmul(pb[:], w, x_s[i][:, 512:1024],
                             start=(i == 0), stop=(i == IC - 1))
        nc.vector.tensor_scalar_add(Qp[og][:, 0:512], pa[:], bqc[:, og:og + 1])
        nc.vector.tensor_scalar_add(Qp[og][:, 512:1024], pb[:],
                                    bqc[:, og:og + 1])

    # ---- K projection ----
    k_pool = {}
    kt_slabs = {}

    def open_k_pool():
        ap = tc.alloc_tile_pool(name="k_pool", bufs=1)
        k_pool["pool"] = ap
        k_pool["x"] = [ap.tile([128, KTOK], BF, name=f"k_x{i}", tag=f"kx{i}")
                       for i in range(IC)]
        k_pool["w"] = [ap.tile([128, H], BF, name=f"k_w{i}", tag=f"kw{i}")
                       for i in range(IC)]

    def k_dma_x(th, ics=range(IC)):
        cs = slice(th * 1024, (th + 1) * 1024)
        for i in ics:
            nc.sync.dma_start(k_pool["x"][i][:, cs],
                              io["kT"][i * 128:(i + 1) * 128, cs])

    def k_dma_w(i):
        nc.sync.dma_start(k_pool["w"][i][:],
                          io["wkT"][i * 128:(i + 1) * 128, :])

    k_acc = {}

    def emit_k_og(og, tp, qr):
        x_s, w_s = k_pool["x"], k_pool["w"]
        if og not in kt_slabs:
            kt_slabs[og] = attnp.tile([128, KTOK], BF, name=f"ktsl{og}",
                                      tag="ktsl", bufs=2)
        if qr == 0:
            pa = psum.tile([128, 512], F32, name="ps_ka", tag="proj")
            pb = psum.tile([128, 512], F32, name="ps_kb", tag="proj")
            k_acc[(og, tp)] = (pa, pb)
        else:
            pa, pb = k_acc[(og, tp)]
        for i in range(qr * 2, qr * 2 + 2):
            w = w_s[i][:, og * 128:(og + 1) * 128]
            nc.tensor.matmul(pa[:], w, x_s[i][:, tp * 1024:tp * 1024 + 512],
                             start=(i == 0), stop=(i == IC - 1))
            nc.tensor.matmul(pb[:], w,
                             x_s[i][:, tp * 1024 + 512:tp * 1024 + 1024],
                             start=(i == 0), stop=(i == IC - 1))
        if qr == 3:
            del k_acc[(og, tp)]
            for ps, tg in ((pa, 0), (pb, 1)):
                col = tp * 1024 + tg * 512
                nc.vector.tensor_scalar_add(
                    kt_slabs[og][:, col:col + 512], ps[:], bkc[:, og:og + 1])

    # ---- V projection: per-quarter double-buffered output tiles ----
    v_pool = {}
    Vq_tiles = {}

    def open_v_pool():
        ap = tc.alloc_tile_pool(name="v_pool", bufs=1)
        v_pool["pool"] = ap

    def v_dma_bv():
        bv_s = v_pool["pool"].tile([1, VW], BF, name="v_b", tag="vb")
        nc.sync.dma_start(bv_s[:], io["bv"][:])
        v_pool["b"] = bv_s

    def v_dma_w(vq):
        o0 = vq * VQ
        w_s = []
        for i in range(IC):
            w = v_pool["pool"].tile([128, VQ], BF, name=f"v_w{i}",
                                    tag=f"vw{i}", bufs=2)
            nc.sync.dma_start(w[:], io["wvT"][i * 128:(i + 1) * 128,
                                              o0:o0 + VQ])
            w_s.append(w)
        v_pool["w"] = w_s

    def v_load_x(th):
        x_s = []
        for i in range(IC):
            x = v_pool["pool"].tile([128, TOK], BF, name=f"v_x{i}",
                                    tag=f"vx{i}", bufs=1)
            nc.sync.dma_start(x[:], io["vT"][i * 128:(i + 1) * 128,
                                             th * TOK:(th + 1) * TOK])
            x_s.append(x)
        v_pool["x"] = x_s

    def emit_v_half(vq, th, tp, hb):
        x_s, wv_s, bv_s = v_pool["x"], v_pool["w"], v_pool["b"]
        o0 = vq * VQ
        kk = th * QT + 2 * tp + hb
        ps = psum.tile([128, VQ], F32, name="ps_v", tag="proj")
        for i in range(IC):
            nc.tensor.matmul(ps[:],
                             x_s[i][:, (2 * tp + hb) * 128:(2 * tp + hb + 1) * 128],
                             wv_s[i][:, 0:VQ], start=(i == 0), stop=False)
        nc.tensor.matmul(ps[:], ones[0:1, 0:128], bv_s[0:1, o0:o0 + VQ],
                         start=False, stop=True)
        vt = attnp.tile([128, VQ], BF, name=f"Vq{kk}", tag=f"Vq{kk}", bufs=2)
        nc.vector.tensor_scalar_mul(vt[:], ps[:], emask[:, kk:kk + 1])
        Vq_tiles[(vq, kk)] = vt

    def emit_v_tp(vq, th, tps):
        for tp in tps:
            emit_v_half(vq, th, tp, 0)
            emit_v_half(vq, th, tp, 1)

    # ---- attention ----
    E_tiles = {}

    def emit_qkt(p, qg, kcs):
        kt = kt_slabs[p]
        cs = slice(qg * 512, (qg + 1) * 512)
        for kc in kcs:
            ps = psum.tile([128, TOK], F32, name="ps_s", tag="scores")
            nc.tensor.matmul(ps[:, 0:512], kt[0:64, kc * 128:(kc + 1) * 128],
                             Qp[p][0:64, cs], start=True, stop=True)
            nc.tensor.matmul(ps[:, 512:1024],
                             kt[64:128, kc * 128:(kc + 1) * 128],
                             Qp[p][64:128, cs], start=True, stop=True)
            e = attnp.tile([128, TOK], BF, name=f"E{kc}", tag="E", bufs=E_BUFS)
            nc.scalar.activation(e[:], ps[:], Exp, bias=nln8[:], scale=1.0)
            E_tiles[(p, qg, kc)] = e

    pv_state = {}

    def emit_pv_begin(p, qg, chains=None, btag_only=False):
        chs = chains if chains is not None else CH
        if btag_only:
            blk = psum.tile([128, 512], F32, name="ps_cb", tag="ctxb", bufs=1)
            slots = [(blk[:, (2 * i) * 128:(2 * i) * 128 + 65],
                      blk[:, (2 * i + 1) * 128:(2 * i + 1) * 128 + 65])
                     for i in range(len(chs))]
        else:
            blka = psum.tile([128, 512], F32, name="ps_ca", tag="ctxa", bufs=1)
            blkb = psum.tile([128, 512], F32, name="ps_cb", tag="ctxb", bufs=1)
            slots = [(blka[:, i * 128:i * 128 + 65],
                      blkb[:, i * 128:i * 128 + 65]) for i in range(len(chs))]
        st = []
        for ci, ((hb, jp), (pa, pb)) in enumerate(zip(chs, slots)):
            h = 2 * p + hb
            vs = slice((h % 4) * 65, (h % 4) * 65 + 65)
            base = hb * 512
            qa, qb = 2 * jp, 2 * jp + 1
            # start=True clears has_written for the whole PSUM bank, so
            # only the first chain touching each bank may issue it; later
            # chains' first write overwrites via the cleared bits.
            sa = ci == 0
            sb = False if btag_only else ci == 0
            st.append((h, vs, base, qa, qb, pa, pb, sa, sb))
        pv_state[(p, qg)] = st

    def emit_pv_steps(p, qg, kcs):
        vq = p // 2
        st = pv_state[(p, qg)]
        for kc in kcs:
            e = E_tiles[(p, qg, kc)]
            vt = Vq_tiles[(vq, kc)]
            for (h, vs, base, qa, qb, pa, pb, sa, sb) in st:
                nc.tensor.matmul(pa,
                                 e[:, base + qa * 128:base + (qa + 1) * 128],
                                 vt[:, vs],
                                 start=(kc == 0 and sa),
                                 stop=(kc == KC - 1), skip_group_check=True)
                nc.tensor.matmul(pb,
                                 e[:, base + qb * 128:base + (qb + 1) * 128],
                                 vt[:, vs],
                                 start=(kc == 0 and sb),
                                 stop=(kc == KC - 1), skip_group_check=True)
            if kc == KC - 1:
                for (h, vs, base, qa, qb, pa, pb, sa, sb) in st:
                    for ps, j in ((pa, qa), (pb, qb)):
                        qt = qg * 4 + j
                        rec = attnp.tile([128, 1], F32, name="rec", tag="rec",
                                         bufs=4)
                        nc.vector.reciprocal(rec[:], ps[:, 64:65])
                        nc.vector.tensor_scalar_mul(
                            ctx[qt][:, h * 64:(h + 1) * 64], ps[:, 0:64],
                            rec[:])
                del pv_state[(p, qg)]

    # ---- output projection ----
    o_pool = {}

    def open_o_pool():
        ap = tc.alloc_tile_pool(name="o_pool", bufs=1)
        o_pool["pool"] = ap
        o_pool["w"] = []
        for i in range(IC):
            w = ap.tile([128, H], BF, name=f"o_w{i}", tag=f"ow{i}")
            nc.sync.dma_start(w[:], io["woT"][i * 128:(i + 1) * 128, :])
            o_pool["w"].append(w)
        bo_s = ap.tile([1, H], BF, name="o_b", tag="ob")
        nc.sync.dma_start(bo_s[:], io["bo"][:])
        o_pool["b"] = bo_s

    def emit_o_tt(tt):
        cp, wo_s, bo_s = o_pool["pool"], o_pool["w"], o_pool["b"]
        osb = cp.tile([128, H], F32, name="osb", tag="osb", bufs=2)
        pa = psum.tile([128, 512], F32, name="ps_oa", tag="proj")
        pb = psum.tile([128, 512], F32, name="ps_ob", tag="proj")
        trs = [None] * IC

        def transpose_i(i):
            ps_t = psum.tile([128, 128], BF, name="ps_t", tag="ctxa",
                              bufs=1)
            nc.tensor.transpose(
                ps_t[:], ctx[tt][:, i * 128:(i + 1) * 128], ident[:])
            tr = cp.tile([128, 128], BF, name="tr", tag="tr", bufs=4)
            nc.vector.tensor_copy(tr[:], ps_t[:])
            trs[i] = tr

        transpose_i(0)
        transpose_i(1)
        for i in range(IC):
            if i + 2 < IC:
                transpose_i(i + 2)
            nc.tensor.matmul(pa[:], trs[i][:], wo_s[i][:, 0:512],
                             start=(i == 0), stop=False)
            nc.tensor.matmul(pb[:], trs[i][:], wo_s[i][:, 512:1024],
                             start=(i == 0), stop=False)
        nc.tensor.matmul(pa[:], ones[0:1, 0:128], bo_s[0:1, 0:512],
                         start=False, stop=True)
        nc.tensor.matmul(pb[:], ones[0:1, 0:128], bo_s[0:1, 512:1024],
                         start=False, stop=True)
        nc.vector.tensor_copy(osb[:, 0:512], pa[:])
        nc.vector.tensor_copy(osb[:, 512:1024], pb[:])
        nc.sync.dma_start(io["out"][tt * 128:(tt + 1) * 128, :], osb[:])

    # ---- emission schedule ----
    CH = [(0, 0), (1, 0), (0, 1), (1, 1)]

    open_k_pool()
    open_v_pool()
    open_q_pool()          # interleaved full qx/qw DMAs
    for i in range(IC):    # kx th0 + full kw, interleaved per ic
        k_dma_x(0, (i,))
        k_dma_w(i)
    k_dma_x(1)
    v_dma_bv()
    v_dma_w(0)
    v_load_x(0)

    # Phase engine: 16 qkt slots; pv steps trail by ~2 slots; the final
    # steps (kc12-15) of a phase are carried into the next phase's head
    # so no non-qkt work sits between the last exp feed and the next
    # phase's qkt.  weave = 4 work callables spliced at even points.
    carry = []

    def do_carry():
        for w in carry:
            w()
        carry.clear()

    def phase(p, qg, weave):
        """16 qkt+exp slots; pv steps trail ~2 slots; weave = list of up
        to 8 work callables, one per 2-slot group."""
        w = list(weave) + [nop] * (8 - len(weave))
        emit_pv_begin(p, qg)
        emit_qkt(p, qg, range(0, 2))
        do_carry()
        w[0]()
        emit_qkt(p, qg, range(2, 4))
        emit_pv_steps(p, qg, range(0, 2))
        w[1]()
        emit_qkt(p, qg, range(4, 6))
        emit_pv_steps(p, qg, range(2, 4))
        w[2]()
        emit_qkt(p, qg, range(6, 8))
        emit_pv_steps(p, qg, range(4, 6))
        w[3]()
        emit_qkt(p, qg, range(8, 10))
        emit_pv_steps(p, qg, range(6, 8))
        w[4]()
        emit_qkt(p, qg, range(10, 12))
        emit_pv_steps(p, qg, range(8, 10))
        w[5]()
        emit_qkt(p, qg, range(12, 14))
        emit_pv_steps(p, qg, range(10, 12))
        w[6]()
        emit_qkt(p, qg, range(14, 16))
        emit_pv_steps(p, qg, range(12, 14))
        w[7]()
        carry.append(lambda: emit_pv_steps(p, qg, range(14, 16)))

    def nop():
        pass

    def emit_k_tp(og, tp):
        for qr in range(4):
            emit_k_og(og, tp, qr)

    # ---- pair 0 (warmup): V quarter 0 lands just-in-time, pv steps
    # trail it; K og1 projected in qg1 ----
    emit_q_og(0)
    emit_k_tp(0, 0)
    emit_pv_begin(0, 0)
    emit_qkt(0, 0, range(0, 2))
    emit_q_og(1)
    emit_qkt(0, 0, range(2, 4))
    emit_k_og(0, 1, 0)
    emit_k_og(0, 1, 1)
    emit_qkt(0, 0, range(4, 6))
    emit_k_og(0, 1, 2)
    emit_k_og(0, 1, 3)
    emit_qkt(0, 0, range(6, 8))
    emit_v_half(0, 0, 0, 0)
    emit_v_half(0, 0, 0, 1)
    emit_pv_steps(0, 0, range(0, 2))
    emit_qkt(0, 0, range(8, 10))
    emit_v_half(0, 0, 1, 0)
    emit_v_half(0, 0, 1, 1)
    emit_pv_steps(0, 0, range(2, 4))
    emit_qkt(0, 0, range(10, 12))
    emit_v_half(0, 0, 2, 0)
    emit_v_half(0, 0, 2, 1)
    emit_pv_steps(0, 0, range(4, 6))
    emit_qkt(0, 0, range(12, 14))
    emit_v_half(0, 0, 3, 0)
    emit_v_half(0, 0, 3, 1)
    emit_pv_steps(0, 0, range(6, 8))
    emit_qkt(0, 0, range(14, 16))
    v_load_x(1)
    # qg1
    emit_pv_begin(0, 1)
    emit_qkt(0, 1, range(0, 2))
    emit_v_half(0, 1, 0, 0)
    emit_v_half(0, 1, 0, 1)
    emit_qkt(0, 1, range(2, 4))
    emit_v_half(0, 1, 1, 0)
    emit_v_half(0, 1, 1, 1)
    emit_pv_steps(0, 0, range(8, 10))
    emit_qkt(0, 1, range(4, 6))
    emit_v_half(0, 1, 2, 0)
    emit_v_half(0, 1, 2, 1)
    emit_pv_steps(0, 0, range(10, 12))
    emit_qkt(0, 1, range(6, 8))
    emit_v_half(0, 1, 3, 0)
    emit_v_half(0, 1, 3, 1)
    emit_pv_steps(0, 0, range(12, 14))
    emit_qkt(0, 1, range(8, 10))
    emit_k_og(1, 0, 0)
    emit_k_og(1, 0, 1)
    emit_pv_steps(0, 0, range(14, 16))
    emit_qkt(0, 1, range(10, 12))
    emit_k_og(1, 0, 2)
    emit_k_og(1, 0, 3)
    emit_pv_steps(0, 1, range(0, 2))
    emit_qkt(0, 1, range(12, 14))
    emit_k_og(1, 1, 0)
    emit_k_og(1, 1, 1)
    emit_pv_steps(0, 1, range(2, 6))
    emit_qkt(0, 1, range(14, 16))
    emit_k_og(1, 1, 2)
    emit_k_og(1, 1, 3)
    emit_pv_steps(0, 1, range(6, 12))
    carry.append(lambda: emit_pv_steps(0, 1, range(12, 16)))

    # ---- pair 1: quarter-1 th0 + K og2 + Q og2/og3 ----
    v_dma_w(1)

    def w_v1(th, tp, hb, load=False):
        def f():
            if load:
                v_load_x(th)
            emit_v_half(1, th, tp, hb)
        return f
    phase(1, 0, [w_v1(0, 0, 0, load=True), w_v1(0, 0, 1),
                 w_v1(0, 1, 0), w_v1(0, 1, 1),
                 lambda: emit_k_og(2, 0, 0), lambda: emit_k_og(2, 0, 1),
                 lambda: emit_k_og(2, 0, 2), lambda: emit_k_og(2, 0, 3)])
    phase(1, 1, [w_v1(0, 2, 0), w_v1(0, 2, 1),
                 w_v1(0, 3, 0), w_v1(0, 3, 1),
                 lambda: emit_k_og(2, 1, 0), lambda: emit_k_og(2, 1, 1),
                 lambda: emit_k_og(2, 1, 2),
                 lambda: (emit_k_og(2, 1, 3), emit_q_og(2))])

    o_acc = {}

    def emit_o_main(tt, part):
        """Accumulate ctx[tt] chunks i=0..6 (pairs 0-6, all complete before
        pair 7) into the O projection; evacuate the partial to SBUF so the
        proj PSUM pair frees for the next tile's main."""
        cp, wo_s = o_pool["pool"], o_pool["w"]
        if part == 0:
            osb = cp.tile([128, H], F32, name="osb", tag="osb", bufs=4)
            pa = psum.tile([128, 512], F32, name="ps_oa", tag="proj")
            pb = psum.tile([128, 512], F32, name="ps_ob", tag="proj")
            blkT = psum.tile([128, 1024], BF, name="ps_t", tag="ctxa", bufs=1)
            trs = [None] * IC
            o_acc[tt] = (osb, pa, pb, blkT, trs)
            rng = range(0, 4)
        else:
            osb, pa, pb, blkT, trs = o_acc[tt]
            rng = range(4, 7)
        for i in rng:
            sl = blkT[:, (i % 8) * 128:(i % 8) * 128 + 128]
            nc.tensor.transpose(sl, ctx[tt][:, i * 128:(i + 1) * 128],
                                ident[:])
            tr = cp.tile([128, 128], BF, name="tr", tag="tr", bufs=8)
            nc.vector.tensor_copy(tr[:], sl)
            trs[i] = tr
        for i in rng:
            nc.tensor.matmul(pa[:], trs[i][:], wo_s[i][:, 0:512],
                             start=(i == 0), stop=(i == 6))
            nc.tensor.matmul(pb[:], trs[i][:], wo_s[i][:, 512:1024],
                             start=(i == 0), stop=(i == 6))
        if part == 1:
            nc.vector.tensor_copy(osb[:, 0:512], pa[:])
            nc.vector.tensor_copy(osb[:, 512:1024], pb[:])

    def emit_o_fin(tt):
        """Chunk 7 (pair 7's heads) + bias, added to the SBUF partial."""
        cp, wo_s, bo_s = o_pool["pool"], o_pool["w"], o_pool["b"]
        osb, pa2, pb2, blkT, trs = o_acc.pop(tt)
        i = IC - 1
        pa = psum.tile([128, 512], F32, name="ps_oa", tag="proj")
        pb = psum.tile([128, 512], F32, name="ps_ob", tag="proj")
        blkT2 = psum.tile([128, 1024], BF, name="ps_t", tag="ctxa", bufs=1)
        sl = blkT2[:, 0:128]
        nc.tensor.transpose(sl, ctx[tt][:, i * 128:(i + 1) * 128], ident[:])
        tr = cp.tile([128, 128], BF, name="tr", tag="tr", bufs=8)
        nc.vector.tensor_copy(tr[:], sl)
        nc.tensor.matmul(pa[:], tr[:], wo_s[i][:, 0:512],
                         start=True, stop=False)
        nc.tensor.matmul(pb[:], tr[:], wo_s[i][:, 512:1024],
                         start=True, stop=False)
        nc.tensor.matmul(pa[:], ones[0:1, 0:128], bo_s[0:1, 0:512],
                         start=False, stop=True)
        nc.tensor.matmul(pb[:], ones[0:1, 0:128], bo_s[0:1, 512:1024],
                         start=False, stop=True)
        nc.vector.tensor_add(osb[:, 0:512], osb[:, 0:512], pa[:])
        nc.vector.tensor_add(osb[:, 512:1024], osb[:, 512:1024], pb[:])
        nc.sync.dma_start(io["out"][tt * 128:(tt + 1) * 128, :], osb[:])

    # ---- pairs 2..6: quarter th1 split over even-pair qg0 weaves 0-1;
    # th0 of the next quarter over odd-pair qg1 weaves 0-1; K og(p+1)
    # at qg0 weave 2 / qg1 weave 2 ----
    def w_v(vq, th, tps, load=False):
        def f():
            if load:
                v_load_x(th)
            emit_v_tp(vq, th, tps)
        return f

    def w_vh(vq, th, tp, hb, load=False):
        def f():
            if load:
                v_load_x(th)
            emit_v_half(vq, th, tp, hb)
        return f

    def w_k(og, tp, qr):
        return lambda: emit_k_og(og, tp, qr)

    for p in range(2, 7):
        qog = {2: 3, 3: 4, 4: 5, 5: 6, 6: 7}[p]
        if p % 2 == 0:
            vq, th = p // 2, 1
            w0 = [w_vh(vq, th, 0, 0, load=True), w_vh(vq, th, 0, 1),
                  w_vh(vq, th, 1, 0), w_vh(vq, th, 1, 1),
                  w_vh(vq, th, 2, 0), w_vh(vq, th, 2, 1),
                  w_vh(vq, th, 3, 0), w_vh(vq, th, 3, 1)]
            w1 = [w_k(p + 1, 0, 0), w_k(p + 1, 0, 1),
                  w_k(p + 1, 0, 2), w_k(p + 1, 0, 3),
                  w_k(p + 1, 1, 0), w_k(p + 1, 1, 1),
                  w_k(p + 1, 1, 2),
                  lambda p=p, qog=qog: (emit_k_og(p + 1, 1, 3),
                                        emit_q_og(qog))]
        else:
            vq, th = (p + 1) // 2, 0
            v_dma_w(vq)
            w0 = [w_vh(vq, th, 0, 0, load=True), w_vh(vq, th, 0, 1),
                  w_vh(vq, th, 1, 0), w_vh(vq, th, 1, 1),
                  w_k(p + 1, 0, 0), w_k(p + 1, 0, 1),
                  w_k(p + 1, 0, 2), w_k(p + 1, 0, 3)]
            w1 = [w_vh(vq, th, 2, 0), w_vh(vq, th, 2, 1),
                  w_vh(vq, th, 3, 0), w_vh(vq, th, 3, 1),
                  w_k(p + 1, 1, 0), w_k(p + 1, 1, 1),
                  w_k(p + 1, 1, 2),
                  lambda p=p, qog=qog: (emit_k_og(p + 1, 1, 3),
                                        emit_q_og(qog))]
        phase(p, 0, w0)
        phase(p, 1, w1)
        if p == 6:
            q_pool["pool"].release()
            v_pool["pool"].release()
            k_pool["pool"].release()
            open_o_pool()

    # ---- pair 7: pv on ctxb only (CH halves, second half re-passes E);
    # O-proj mains (chunks 0-6) fill the idle, fins (chunk 7) trail ----
    emit_pv_begin(7, 0, chains=CH[:2], btag_only=True)
    emit_qkt(7, 0, range(0, 2))
    do_carry()
    emit_qkt(7, 0, range(2, 4))
    emit_pv_steps(7, 0, range(0, 2))
    emit_o_main(0, 0)
    emit_qkt(7, 0, range(4, 6))
    emit_pv_steps(7, 0, range(2, 4))
    emit_o_main(0, 1)
    emit_qkt(7, 0, range(6, 8))
    emit_pv_steps(7, 0, range(4, 6))
    emit_o_main(1, 0)
    emit_qkt(7, 0, range(8, 10))
    emit_pv_steps(7, 0, range(6, 8))
    emit_o_main(1, 1)
    emit_qkt(7, 0, range(10, 12))
    emit_pv_steps(7, 0, range(8, 10))
    emit_o_main(2, 0)
    emit_qkt(7, 0, range(12, 14))
    emit_pv_steps(7, 0, range(10, 12))
    emit_o_main(2, 1)
    emit_qkt(7, 0, range(14, 16))
    emit_pv_steps(7, 0, range(12, 16))
    emit_pv_begin(7, 0, chains=CH[2:], btag_only=True)
    emit_pv_steps(7, 0, range(0, 16))     # re-pass over live E(7,0)
    # qg1
    emit_pv_begin(7, 1, chains=CH[:2], btag_only=True)
    emit_qkt(7, 1, range(0, 2))
    emit_o_main(3, 0)
    emit_qkt(7, 1, range(2, 4))
    emit_pv_steps(7, 1, range(0, 2))
    emit_o_main(3, 1)
    emit_qkt(7, 1, range(4, 6))
    emit_pv_steps(7, 1, range(2, 4))
    emit_o_fin(0)
    emit_qkt(7, 1, range(6, 8))
    emit_pv_steps(7, 1, range(4, 6))
    emit_o_main(4, 0)
    emit_qkt(7, 1, range(8, 10))
    emit_pv_steps(7, 1, range(6, 8))
    emit_o_main(4, 1)
    emit_qkt(7, 1, range(10, 12))
    emit_pv_steps(7, 1, range(8, 10))
    emit_o_fin(1)
    emit_qkt(7, 1, range(12, 14))
    emit_pv_steps(7, 1, range(10, 12))
    emit_o_main(5, 0)
    emit_qkt(7, 1, range(14, 16))
    emit_o_main(5, 1)
    emit_pv_steps(7, 1, range(12, 16))
    emit_pv_begin(7, 1, chains=CH[2:], btag_only=True)
    emit_pv_steps(7, 1, range(0, 8))
    emit_o_fin(2)
    emit_pv_steps(7, 1, range(8, 16))
    emit_o_fin(3)
    emit_o_main(6, 0)
    emit_o_main(6, 1)
    emit_o_fin(4)
    emit_o_fin(5)
    emit_o_main(7, 0)
    emit_o_main(7, 1)
    emit_o_fin(6)
    emit_o_fin(7)

    o_pool["pool"].release()
    attnp.release()
    psum.release()
    persist.release()


def _build():
    nc = bacc.Bacc("TRN2", target_bir_lowering=False, debug=False,
                   num_devices=NCORES)
    io = {}

    def inp(name, shape, dtype=BF):
        io[name] = nc.dram_tensor(name, shape, dtype, kind="ExternalInput").ap()
    inp("qT", [H, TOK])
    inp("kT", [H, KTOK])
    inp("vT", [H, KTOK])
    inp("wqT", [H, H])
    inp("wkT", [H, H])
    inp("wvT", [H, VW])
    inp("woT", [H, H])
    inp("bv", [1, VW])
    inp("bo", [1, H])
    inp("bqcol", [128, IC], F32)
    inp("bkcol", [128, IC], F32)
    inp("emaskcol", [128, KC], F32)
    io["out"] = nc.dram_tensor("out", [TOK, H], F32, kind="ExternalOutput").ap()

    with tile.TileContext(nc) as tc:
        _emit(nc, tc, io)
    nc.compile()
    return nc, io


def get_compiled():
    if "nc" not in _CACHE:
        _CACHE["nc"], _CACHE["io"] = _build()
    return _CACHE["nc"]


def make_in_maps(query, key_, value, attention_mask, Wq, bq, Wk, bk, Wv, bv,
                 Wo, bo):
    bf = ml_dtypes.bfloat16
    f32 = np.float32
    query = np.asarray(query, f32)
    key_ = np.asarray(key_, f32)
    value = np.asarray(value, f32)
    attention_mask = np.asarray(attention_mask, f32)
    Wq, bq = np.asarray(Wq, f32), np.asarray(bq, f32)
    Wk, bk = np.asarray(Wk, f32), np.asarray(bk, f32)
    Wv, bv = np.asarray(Wv, f32), np.asarray(bv, f32)
    Wo, bo = np.asarray(Wo, f32), np.asarray(bo, f32)

    scale = 1.0 / np.sqrt(np.float32(HDIM))
    wqT = np.ascontiguousarray((Wq * scale).T).astype(bf)
    wkT = np.ascontiguousarray(Wk.T).astype(bf)
    woT = np.ascontiguousarray(Wo.T).astype(bf)
    wvT = np.zeros((H, VW), f32)
    bv_ext = np.zeros((1, VW), f32)
    for h in range(NH):
        wvT[:, h * 65:h * 65 + 64] = Wv[h * 64:(h + 1) * 64, :].T
        bv_ext[0, h * 65:h * 65 + 64] = bv[h * 64:(h + 1) * 64]
        bv_ext[0, h * 65 + 64] = 1.0
    wvT = wvT.astype(bf)
    bv_ext = bv_ext.astype(bf)
    bo_s = bo.reshape(1, H).astype(bf)
    bqcol = np.ascontiguousarray((bq * scale).reshape(IC, 128).T).astype(f32)
    bkcol = np.ascontiguousarray(bk.reshape(IC, 128).T).astype(f32)

    in_maps = []
    for c in range(NCORES):
        b, half = divmod(c, 2)
        sl = slice(half * TOK, (half + 1) * TOK)
        qT = np.ascontiguousarray(query[b, sl, :].T).astype(bf)
        kT = np.ascontiguousarray(key_[b].T).astype(bf)
        vT = np.ascontiguousarray(value[b].T).astype(bf)
        with np.errstate(over="ignore", under="ignore"):
            em = np.exp((1.0 - attention_mask[b]) * -10000.0).astype(f32)
        emaskcol = np.ascontiguousarray(em.reshape(KC, 128).T).astype(f32)
        in_maps.append({
            "qT": qT, "kT": kT, "vT": vT,
            "wqT": wqT, "wkT": wkT, "wvT": wvT, "woT": woT,
            "bv": bv_ext, "bo": bo_s,
            "bqcol": bqcol, "bkcol": bkcol,
            "emaskcol": emaskcol,
        })
    return in_maps


def kernel(query, key_, value, attention_mask, Wq, bq, Wk, bk, Wv, bv, Wo, bo,
           **run_kwargs):
    nc = get_compiled()
    in_maps = make_in_maps(query, key_, value, attention_mask, Wq, bq, Wk, bk,
                           Wv, bv, Wo, bo)
    res = run_bass_kernel_spmd(nc, in_maps, core_ids=list(range(NCORES)),
                               **run_kwargs)
    out = np.empty((B, S, H), np.float32)
    for c in range(NCORES):
        b, half = divmod(c, 2)
        out[b, half * TOK:(half + 1) * TOK, :] = res.results[c]["out"]
    if run_kwargs:
        kernel.last_results = res
    return out
